# revision 48
# baseline (speedup 1.0000x reference)
"""Causal multi-head attention (B=2, S=2048, D=1024, H=16, Dh=64) on 8 TRN2 cores.

Sharding: core c -> batch b=c//4, head-group g=c%4 (heads 4g..4g+3, d_out cols
g*256..(g+1)*256). Each core computes Q/K/V projections for its head group from
x[b] and runs causal attention for its 4 heads independently. No collectives.

Per-core dataflow:
  phase A: load x[b]^T (pre-transposed + fp32r-rounded on host) + W slices; PE
           computes Q^T,K^T in fp32r (1 cyc/row) -> stored fp16 (head-pair
           layout [128, S]) and V+ones (fp16, [S, 4*65] interleaved per head).
  phase B: per (q-tile, head): S row chunks = Q_h^T.T @ K_h^T (fp16) into PSUM
           f32 (two heads packed into PE row-groups 0-63 / 64-127); causal mask
           on the diagonal block applied by an extra PE matmul (eye.T @ maskneg
           accumulated into the same PSUM group), row-max (DVE reduce, negated),
           exp(S - max) on ACT (per-partition bias) -> P row fp16, batched
           128-block transpose via DMA xbar into per-(head, q-chunk) k-major
           tiles, then O^T[65, 512] = sum_kt V~[kt].T @ P^T[kt] on PE (fp16,
           row 64 = softmax denominator via the ones column).
  host:    out = (O^T[:64] / O^T[64]) transposed back, assembled across cores.
"""

import math

import numpy as np

B = 2
SEQ = 2048
DIN = 1024
H = 16
DH = 64
NCORES = 8
DO = 256  # d_out columns per core (4 heads)
HPC = 4  # heads per core
KT_N = DIN // 128  # 8 contraction tiles
ST_N = SEQ // 128  # 16 seq tiles
QC_N = SEQ // 512  # 4 q-chunks for PV
NEG = -1.0e9
SUB = 1024  # S-row PSUM subtile length (2 banks)

_CACHE = {}
LAST_RESULTS = None


def _emit_core_kernel(tc, outs, ins):
    from concourse import mybir

    nc = tc.nc
    f32 = mybir.dt.float32
    f32r = mybir.dt.float32r
    f16 = mybir.dt.float16
    (outT,) = outs  # [HPC, 65, SEQ] f16
    xT, wq_hi, wq_lo, wk_hi, wk_lo, wv, mask, eye = ins

    from contextlib import ExitStack

    with ExitStack() as ctx:
        consts = ctx.enter_context(tc.tile_pool(name="consts", bufs=1))
        proj_out = ctx.enter_context(tc.tile_pool(name="proj_out", bufs=1))
        xs_pool = ctx.enter_context(tc.tile_pool(name="xs", bufs=2))
        prow_pool = ctx.enter_context(tc.tile_pool(name="prow", bufs=4))
        ptrow_pool = ctx.enter_context(tc.tile_pool(name="ptrow", bufs=2))
        stats = ctx.enter_context(tc.tile_pool(name="stats", bufs=8))
        outp = ctx.enter_context(tc.tile_pool(name="outp", bufs=3))
        # separate PSUM pools per phase so the projection never stalls on
        # attention-tile WAR hazards: proj 2 banks + S 4 banks + PV 2 banks
        ps_proj = ctx.enter_context(
            tc.tile_pool(name="ps_proj", bufs=2, space="PSUM")
        )
        ps_main = ctx.enter_context(
            tc.tile_pool(name="ps_main", bufs=2, space="PSUM")
        )
        ps_o = ctx.enter_context(tc.tile_pool(name="ps_o", bufs=2, space="PSUM"))

        mask_sb = consts.tile([128, 512], f16, tag="mask")
        nc.gpsimd.dma_start(mask_sb[:], mask[:])
        eye_sb = consts.tile([128, 128], f16, tag="eye")
        nc.gpsimd.dma_start(eye_sb[:], eye[:])
        w_sb = {}
        for wname, wap in (
            ("wq_hi", wq_hi),
            ("wq_lo", wq_lo),
            ("wk_hi", wk_hi),
            ("wk_lo", wk_lo),
            ("wv", wv),
        ):
            t = consts.tile([128, KT_N, DO], f32r, tag=wname, name=f"{wname}_sb")
            nc.gpsimd.dma_start(t[:], wap.rearrange("(k p) n -> p k n", p=128))
            w_sb[wname] = t

        qt_sb = [
            proj_out.tile([128, SEQ], f32r, tag=f"qt{m}", name=f"qt{m}")
            for m in range(2)
        ]
        kt_sb = [
            proj_out.tile([128, SEQ], f32r, tag=f"kt{m}", name=f"kt{m}")
            for m in range(2)
        ]
        v_sb = [
            proj_out.tile([128, HPC, DH + 1], f16, tag=f"v{s}", name=f"v{s}")
            for s in range(ST_N)
        ]
        for st in range(ST_N):
            nc.gpsimd.memset(v_sb[st][:, :, DH : DH + 1], 1.0)

        def proj_fillers(sc):
            """Emission closures for projecting s-chunk sc: xs loads, 4 QK
            matmul groups, 4 V groups. Meant to be spread between S units
            so the PE always has dispatchable work."""
            xs = []

            def load():
                for k in range(KT_N):
                    t = xs_pool.tile([128, 512], f32r, tag=f"xs{k}", name=f"xs{k}")
                    nc.gpsimd.dma_start(
                        t[:],
                        xT[k * 128 : (k + 1) * 128, sc * 512 : (sc + 1) * 512],
                    )
                    xs.append(t)

            fs = [load]

            def qk_group(wname, dst, m):
                pst = ps_proj.tile([128, 512], f32, tag="pst", name="pproj")
                for k in range(KT_N):
                    nc.tensor.matmul(
                        pst[:],
                        w_sb[wname + "_hi"][:, k, m * 128 : (m + 1) * 128],
                        xs[k][:],
                        start=(k == 0),
                        stop=False,
                    )
                for k in range(KT_N):
                    nc.tensor.matmul(
                        pst[:],
                        w_sb[wname + "_lo"][:, k, m * 128 : (m + 1) * 128],
                        xs[k][:],
                        start=False,
                        stop=(k == KT_N - 1),
                    )
                nc.vector.tensor_copy(dst[m][:, sc * 512 : (sc + 1) * 512], pst[:])

            for wname, dst in (("wk", kt_sb), ("wq", qt_sb)):
                for m in range(2):
                    fs.append(
                        lambda wname=wname, dst=dst, m=m: qk_group(wname, dst, m)
                    )

            def v_group(j):
                st = 4 * sc + j
                psvt = ps_proj.tile([128, 512], f32, tag="pst", name="pv")
                psv = psvt[:, 0:DO]
                for k in range(KT_N):
                    nc.tensor.matmul(
                        psv,
                        xs[k][:, j * 128 : (j + 1) * 128],
                        w_sb["wv"][:, k, :],
                        start=(k == 0),
                        stop=(k == KT_N - 1),
                    )
                nc.scalar.copy(
                    v_sb[st][:, :, 0:DH],
                    psv.rearrange("p (h d) -> p h d", h=HPC),
                )

            for j in range(4):
                fs.append(lambda j=j: v_group(j))
            return fs

        def emit_attention_S(qc, fillers=()):
            """S = QK^T, causal mask, softmax numerator P (fp16) + its
            transposes for q-tiles qc*4..qc*4+3, all heads. `fillers` are
            independent emission closures spread between the 16 S units to
            keep the in-order PE queue fed while the DVE/ACT softmax
            pipeline drains."""
            fillers = list(fillers)
            emitted = 0
            unit = 0
            pt_tiles = {}
            for h in range(HPC):
                pt_tiles[h] = ptrow_pool.tile(
                    [128, ST_N, 512], f16, tag=f"pt{h % 2}", name=f"pt{h % 2}"
                )
            for qt in range(qc * 4, qc * 4 + 4):
                L = (qt + 1) * 128
                for h in range(HPC):
                    m2, poff = h // 2, (h % 2) * 64
                    lhsT_q = qt_sb[m2][poff : poff + 64, qt * 128 : (qt + 1) * 128]
                    subs = [(0, min(L, SUB))]
                    if L > SUB:
                        subs.append((SUB, L - SUB))
                    mneg_parts = stats.tile([128, 2], f32, tag="mneg_p", name="mneg_p")
                    ps_tiles = []
                    for si, (off, ls) in enumerate(subs):
                        ps = ps_main.tile([128, SUB], f32, tag="srow", name="srow")
                        ps_tiles.append((ps, off, ls))
                        has_diag = off + ls == L
                        for c0 in range(0, ls, 512):
                            c1 = min(ls, c0 + 512)
                            if has_diag and c1 == ls:
                                # causal mask for the final chunk: seed the
                                # PSUM region with eye.T @ maskneg (triangle
                                # in the last 128 cols of the slice), then
                                # the S matmul accumulates onto it.
                                w = c1 - c0
                                nc.tensor.matmul(
                                    ps[:, c0:c1],
                                    eye_sb[:],
                                    mask_sb[:, 512 - w : 512],
                                    start=True,
                                    stop=False,
                                    skip_group_check=True,
                                )
                            nc.tensor.matmul(
                                ps[:, c0:c1],
                                lhsT_q,
                                kt_sb[m2][poff : poff + 64, off + c0 : off + c1],
                                start=not (has_diag and c1 == ls),
                                stop=True,
                                skip_group_check=True,
                            )
                        nc.vector.reduce_max(
                            mneg_parts[:, si : si + 1],
                            ps[:, :ls],
                            axis=mybir.AxisListType.X,
                            negate=True,
                        )
                    if len(subs) == 2:
                        mneg = stats.tile([128, 1], f32, tag="mneg", name="mneg")
                        nc.vector.tensor_reduce(
                            mneg[:, 0:1],
                            mneg_parts[:, 0:2],
                            axis=mybir.AxisListType.X,
                            op=mybir.AluOpType.min,
                        )
                        mneg_ap = mneg[:, 0:1]
                    else:
                        mneg_ap = mneg_parts[:, 0:1]

                    p_row = prow_pool.tile([128, SEQ], f16, tag="prow", name="prow")
                    for ps, off, ls in ps_tiles:
                        nc.scalar.activation(
                            p_row[:, off : off + ls],
                            ps[:, :ls],
                            mybir.ActivationFunctionType.Exp,
                            bias=mneg_ap,
                            scale=1.0,
                        )
                    # all transposes stay on ONE HWDGE queue: concurrent
                    # dma_start_transpose on two queues corrupts transfers
                    nc.sync.dma_start_transpose(
                        pt_tiles[h][
                            :, : qt + 1, (qt % 4) * 128 : (qt % 4) * 128 + 128
                        ],
                        p_row[:, :L],
                    )
                    unit += 1
                    want = (unit * len(fillers)) // 16
                    while emitted < want:
                        fillers[emitted]()
                        emitted += 1
            while emitted < len(fillers):
                fillers[emitted]()
                emitted += 1
            return pt_tiles

        def pv_head(qc, pt_tiles, h):
            po = ps_o.tile([65, 512], f32, tag="po", name="po")
            kt_hi = qc * 4 + 3
            for kt in range(kt_hi + 1):
                off = max(0, (kt - qc * 4)) * 128
                nc.tensor.matmul(
                    po[:, off:512],
                    v_sb[kt][:, h, :],
                    pt_tiles[h][:, kt, off:512],
                    start=(kt == 0),
                    stop=(kt == kt_hi),
                )
            ot = outp.tile([65, 512], f16, tag="ot", name="ot")
            nc.vector.tensor_copy(ot[:], po[:])
            nc.gpsimd.dma_start(outT[h, :, qc * 512 : (qc + 1) * 512], ot[:])

        def pv_fillers(qc, pt_tiles):
            return [
                lambda h=h: pv_head(qc, pt_tiles, h) for h in range(HPC)
            ]

        # software-pipelined emission: projections run two chunks ahead and
        # PV(i) lands inside S(i+2), spread between S units so the in-order
        # PE queue always has dispatchable matmuls.
        for f in proj_fillers(0):
            f()
        for f in proj_fillers(1):
            f()
        pts0 = emit_attention_S(0, proj_fillers(2))
        pts1 = emit_attention_S(1, proj_fillers(3))
        pts2 = emit_attention_S(2, pv_fillers(0, pts0))
        pts3 = emit_attention_S(3, pv_fillers(1, pts1))
        for f in pv_fillers(2, pts2):
            f()
        for f in pv_fillers(3, pts3):
            f()


def _guard_ldweights(nc):
    """Tile hoists standalone LDWEIGHTS prefetches for 2-byte matmuls but
    leaves the weight-readiness waits on the MATMUL — the LDW would read
    stale weights. Bacc's move_matmul_waits_to_ldweights only moves waits
    beyond the first, so a single-wait matmul still leaves its LDW
    unguarded. Move ALL waits from a matmul onto an immediately-preceding
    ldweights (they are emitted adjacent), which is strictly safe."""
    from concourse import mybir

    n = 0
    for f in nc.m.functions:
        for bb in f.blocks:
            prev = None
            for inst in bb.instructions:
                if (
                    type(inst).__name__ == "InstMatmult"
                    and prev is not None
                    and type(prev).__name__ == "InstLdweights"
                    and prev.engine == inst.engine
                ):
                    si = inst.sync_info
                    waits = list(si.on_wait) if si is not None else []
                    if waits:
                        psi = prev.sync_info
                        pw = list(psi.on_wait) if psi is not None else []
                        pu = list(psi.on_update) if psi is not None else []
                        prev.sync_info = mybir.SyncInfo(
                            on_wait=pw + waits, on_update=pu
                        )
                        inst.sync_info = mybir.SyncInfo(
                            on_wait=[], on_update=list(si.on_update)
                        )
                        n += 1
                prev = inst
    return n


def _split_waits(nc):
    """This container's walrus accepts at most ONE sync-wait per instruction
    on several opcodes ("Too many sync wait commands"). Hoist excess waits
    into standalone InstEventSemaphore instructions on the same engine."""
    from concourse import mybir

    n = 0
    for f in nc.m.functions:
        for bb in f.blocks:
            new = []
            for inst in list(bb.instructions):
                # Matmuls get ALL waits hoisted: walrus emits the fused
                # LDWEIGHTS before the matmul's own waits would fire, so a
                # wait left on the matmul does not guard the stationary
                # operand read. A standalone EventSemaphore before the
                # instruction always does.
                cap = 0 if type(inst).__name__ == "InstMatmult" else 1
                si = inst.sync_info
                waits = list(si.on_wait) if si is not None else []
                if len(waits) > cap:
                    for j, w in enumerate(waits[cap:]):
                        new.append(
                            mybir.InstEventSemaphore(
                                name=f"{inst.name}-w{j}",
                                engine=inst.engine,
                                ins=[],
                                outs=[],
                                sync_info=mybir.SyncInfo(on_wait=[w], on_update=[]),
                            )
                        )
                        n += 1
                    inst.sync_info = mybir.SyncInfo(
                        on_wait=waits[:cap], on_update=list(si.on_update)
                    )
                new.append(inst)
            bb.instructions = new
    return n


def _build_nc():
    import concourse.bass as bass
    import concourse.tile as tile
    from concourse import mybir

    f32r = mybir.dt.float32r
    f16 = mybir.dt.float16
    nc = bass.Bass(
        "TRN2",
        target_bir_lowering=False,
        debug=False,
        num_devices=NCORES,
    )
    xT = nc.dram_tensor("xT", [DIN, SEQ], f32r, kind="ExternalInput").ap()
    wq_hi = nc.dram_tensor("wq_hi", [DIN, DO], f32r, kind="ExternalInput").ap()
    wq_lo = nc.dram_tensor("wq_lo", [DIN, DO], f32r, kind="ExternalInput").ap()
    wk_hi = nc.dram_tensor("wk_hi", [DIN, DO], f32r, kind="ExternalInput").ap()
    wk_lo = nc.dram_tensor("wk_lo", [DIN, DO], f32r, kind="ExternalInput").ap()
    wv = nc.dram_tensor("wv", [DIN, DO], f32r, kind="ExternalInput").ap()
    mask = nc.dram_tensor("mask", [128, 512], f16, kind="ExternalInput").ap()
    eye = nc.dram_tensor("eye", [128, 128], f16, kind="ExternalInput").ap()
    outT = nc.dram_tensor("outT", [HPC, DH + 1, SEQ], f16, kind="ExternalOutput").ap()

    with tile.TileContext(nc) as tc:
        _emit_core_kernel(
            tc, (outT,), (xT, wq_hi, wq_lo, wk_hi, wk_lo, wv, mask, eye)
        )
    _guard_ldweights(nc)
    _split_waits(nc)
    return nc


def make_mask():
    """Additive causal mask, fp16, [128, 512] with the 128x128 triangle in
    the last 128 columns — slicing mask[:, 512-w:512] puts the triangle at
    the end of any w-wide final chunk. -57344 is exactly representable and
    large enough that exp(s - 57344 - max) underflows to 0 for any score
    magnitude in this problem."""
    m = np.zeros((128, 512), dtype=np.float16)
    q = np.arange(128)[:, None]
    k = np.arange(128)[None, :]
    m[:, 384:512] = np.where(k > q, np.float16(-57344.0), np.float16(0.0))
    return m


def round_fp32r(a):
    """Round fp32 to the fp32r (E8M11) grid: RNE at the 12 dropped mantissa
    bits, matching the hardware's fp32_to_fp32r downconversion."""
    u = np.ascontiguousarray(a, dtype=np.float32).view(np.uint32)
    lsb = (u >> 12) & 1
    r = (u + 0x7FF + lsb) & 0xFFFFF000
    return r.view(np.float32)


def shard_inputs(x, W_q, W_k, W_v):
    x = np.asarray(x, dtype=np.float32)
    W_q = np.asarray(W_q, dtype=np.float32)
    W_k = np.asarray(W_k, dtype=np.float32)
    W_v = np.asarray(W_v, dtype=np.float32)
    mask = make_mask()
    eye = np.eye(128, dtype=np.float16)
    scale = 1.0 / math.sqrt(DH)
    wq_s = W_q * scale
    wq_hi = round_fp32r(wq_s)
    wq_lo = round_fp32r(wq_s - wq_hi)
    wk_hi = round_fp32r(W_k)
    wk_lo = round_fp32r(W_k - wk_hi)
    in_maps = []
    for c in range(NCORES):
        b, g = divmod(c, NCORES // B)
        sl = slice(g * DO, (g + 1) * DO)
        in_maps.append(
            {
                "xT": round_fp32r(x[b].T),
                "wq_hi": np.ascontiguousarray(wq_hi[:, sl]),
                "wq_lo": np.ascontiguousarray(wq_lo[:, sl]),
                "wk_hi": np.ascontiguousarray(wk_hi[:, sl]),
                "wk_lo": np.ascontiguousarray(wk_lo[:, sl]),
                "wv": round_fp32r(W_v[:, sl]),
                "mask": mask,
                "eye": eye,
            }
        )
    return in_maps


def assemble_output(results):
    out = np.zeros((B, SEQ, DIN), dtype=np.float32)
    for c in range(NCORES):
        b, g = divmod(c, NCORES // B)
        oT = np.asarray(results[c]["outT"], dtype=np.float32)  # [HPC, 65, SEQ]
        for h in range(HPC):
            col = g * DO + h * DH
            out[b, :, col : col + DH] = (oT[h, :DH, :] / oT[h, DH : DH + 1, :]).T
    return out


def _install_axon_ntff_hook():
    """Provide antenv.axon_hooks (missing in this image) so trace=True works
    under axon. Mirrors trn_agent_boot.trn_boot._ntff_profile_via_ctypes."""
    import contextlib
    import ctypes
    import sys
    import types

    if "antenv.axon_hooks" in sys.modules:
        return True
    try:
        lib = ctypes.CDLL("/opt/axon/libaxon_pjrt.so")
    except OSError:
        return False
    if not hasattr(lib, "axon_start_nrt_profile"):
        return False
    lib.axon_start_nrt_profile.argtypes = [
        ctypes.POINTER(ctypes.c_int64),
        ctypes.c_size_t,
    ]
    lib.axon_start_nrt_profile.restype = ctypes.c_int64
    lib.axon_stop_nrt_profile.argtypes = [ctypes.c_char_p]
    lib.axon_stop_nrt_profile.restype = ctypes.c_int64

    @contextlib.contextmanager
    def _hook(output_dir, device_ids):
        import jax

        jax.devices()
        if device_ids:
            ids = (ctypes.c_int64 * len(device_ids))(*device_ids)
            rc = lib.axon_start_nrt_profile(ids, len(device_ids))
        else:
            rc = lib.axon_start_nrt_profile(None, 0)
        if rc != 0:
            raise RuntimeError(f"axon_start_nrt_profile rc={rc}")
        try:
            yield
        finally:
            n = lib.axon_stop_nrt_profile(str(output_dir).encode())
            print(f"ntff profile: {n} file(s) written to {output_dir}")

    mod = types.ModuleType("antenv.axon_hooks")
    holder = [_hook]
    mod.get_axon_ntff_profile_hook = lambda: holder[0]
    mod.set_axon_ntff_profile_hook = lambda h: holder.__setitem__(0, h)
    sys.modules["antenv.axon_hooks"] = mod
    import antenv

    antenv.axon_hooks = mod
    return True


def kernel(x, W_q, W_k, W_v):
    global LAST_RESULTS
    import os

    import concourse.bass_utils as bass_utils
    from concourse.bass_utils import run_bass_kernel_spmd

    if "nc" not in _CACHE:
        _CACHE["nc"] = _build_nc()
    nc = _CACHE["nc"]

    in_maps = shard_inputs(x, W_q, W_k, W_v)

    trace = bool(int(os.environ.get("MHA_TRACE", "0")))
    if trace:
        trace = _install_axon_ntff_hook()
        # avoid the fish-bucket artifact upload in this container
        bass_utils.upload_artifacts = lambda d: str(d)
    res = run_bass_kernel_spmd(
        nc, in_maps, core_ids=list(range(NCORES)), trace=trace
    )
    LAST_RESULTS = res
    return assemble_output(res.results)



# revision 51
# speedup vs baseline: 1.0168x; 1.0168x over previous
"""Causal multi-head attention (B=2, S=2048, D=1024, H=16, Dh=64) on 8 TRN2 cores.

Sharding: core c -> batch b=c//4, head-group g=c%4 (heads 4g..4g+3, d_out cols
g*256..(g+1)*256). Each core computes Q/K/V projections for its head group from
x[b] and runs causal attention for its 4 heads independently. No collectives.

Per-core dataflow:
  phase A: load x[b]^T (pre-transposed + fp32r-rounded on host) + W slices; PE
           computes Q^T,K^T in fp32r (1 cyc/row) -> stored fp16 (head-pair
           layout [128, S]) and V+ones (fp16, [S, 4*65] interleaved per head).
  phase B: per (q-tile, head): S row chunks = Q_h^T.T @ K_h^T (fp16) into PSUM
           f32 (two heads packed into PE row-groups 0-63 / 64-127); causal mask
           on the diagonal block applied by an extra PE matmul (eye.T @ maskneg
           accumulated into the same PSUM group), row-max (DVE reduce, negated),
           exp(S - max) on ACT (per-partition bias) -> P row fp16, batched
           128-block transpose via DMA xbar into per-(head, q-chunk) k-major
           tiles, then O^T[65, 512] = sum_kt V~[kt].T @ P^T[kt] on PE (fp16,
           row 64 = softmax denominator via the ones column).
  host:    out = (O^T[:64] / O^T[64]) transposed back, assembled across cores.
"""

import math

import numpy as np

B = 2
SEQ = 2048
DIN = 1024
H = 16
DH = 64
NCORES = 8
DO = 256  # d_out columns per core (4 heads)
HPC = 4  # heads per core
KT_N = DIN // 128  # 8 contraction tiles
ST_N = SEQ // 128  # 16 seq tiles
QC_N = SEQ // 512  # 4 q-chunks for PV
NEG = -1.0e9
SUB = 1024  # S-row PSUM subtile length (2 banks)

_CACHE = {}
LAST_RESULTS = None


def _emit_core_kernel(tc, outs, ins):
    from concourse import mybir

    nc = tc.nc
    f32 = mybir.dt.float32
    f32r = mybir.dt.float32r
    f16 = mybir.dt.float16
    (outT,) = outs  # [HPC, 65, SEQ] f16
    xT, wq_hi, wq_lo, wk_hi, wk_lo, wv, mask, eye = ins

    from contextlib import ExitStack

    with ExitStack() as ctx:
        consts = ctx.enter_context(tc.tile_pool(name="consts", bufs=1))
        proj_out = ctx.enter_context(tc.tile_pool(name="proj_out", bufs=1))
        xs_pool = ctx.enter_context(tc.tile_pool(name="xs", bufs=2))
        prow_pool = ctx.enter_context(tc.tile_pool(name="prow", bufs=4))
        ptrow_pool = ctx.enter_context(tc.tile_pool(name="ptrow", bufs=2))
        stats = ctx.enter_context(tc.tile_pool(name="stats", bufs=8))
        outp = ctx.enter_context(tc.tile_pool(name="outp", bufs=3))
        # separate PSUM pools per phase so the projection never stalls on
        # attention-tile WAR hazards: proj 2 banks + S 4 banks + PV 2 banks
        ps_proj = ctx.enter_context(
            tc.tile_pool(name="ps_proj", bufs=2, space="PSUM")
        )
        ps_main = ctx.enter_context(
            tc.tile_pool(name="ps_main", bufs=2, space="PSUM")
        )
        ps_o = ctx.enter_context(tc.tile_pool(name="ps_o", bufs=2, space="PSUM"))

        mask_sb = consts.tile([128, 512], f16, tag="mask")
        nc.gpsimd.dma_start(mask_sb[:], mask[:])
        eye_sb = consts.tile([128, 128], f16, tag="eye")
        nc.gpsimd.dma_start(eye_sb[:], eye[:])
        w_sb = {}
        for wname, wap in (
            ("wq_hi", wq_hi),
            ("wq_lo", wq_lo),
            ("wk_hi", wk_hi),
            ("wk_lo", wk_lo),
            ("wv", wv),
        ):
            t = consts.tile([128, KT_N, DO], f32r, tag=wname, name=f"{wname}_sb")
            nc.gpsimd.dma_start(t[:], wap[:])
            w_sb[wname] = t

        qt_sb = [
            proj_out.tile([128, SEQ], f32r, tag=f"qt{m}", name=f"qt{m}")
            for m in range(2)
        ]
        kt_sb = [
            proj_out.tile([128, SEQ], f32r, tag=f"kt{m}", name=f"kt{m}")
            for m in range(2)
        ]
        v_sb = [
            proj_out.tile([128, HPC, DH + 1], f16, tag=f"v{s}", name=f"v{s}")
            for s in range(ST_N)
        ]
        for st in range(ST_N):
            nc.gpsimd.memset(v_sb[st][:, :, DH : DH + 1], 1.0)

        def proj_fillers(sc):
            """Emission closures for projecting s-chunk sc: xs loads, 4 QK
            matmul groups, 4 V groups. Meant to be spread between S units
            so the PE always has dispatchable work."""
            xs = []

            def load():
                for k in range(KT_N):
                    t = xs_pool.tile([128, 512], f32r, tag=f"xs{k}", name=f"xs{k}")
                    nc.gpsimd.dma_start(
                        t[:],
                        xT[k * 128 : (k + 1) * 128, sc * 512 : (sc + 1) * 512],
                    )
                    xs.append(t)

            fs = [load]

            def qk_group(wname, dst, m):
                pst = ps_proj.tile([128, 512], f32, tag="pst", name="pproj")
                for k in range(KT_N):
                    nc.tensor.matmul(
                        pst[:],
                        w_sb[wname + "_hi"][:, k, m * 128 : (m + 1) * 128],
                        xs[k][:],
                        start=(k == 0),
                        stop=False,
                    )
                for k in range(KT_N):
                    nc.tensor.matmul(
                        pst[:],
                        w_sb[wname + "_lo"][:, k, m * 128 : (m + 1) * 128],
                        xs[k][:],
                        start=False,
                        stop=(k == KT_N - 1),
                    )
                nc.vector.tensor_copy(dst[m][:, sc * 512 : (sc + 1) * 512], pst[:])

            for wname, dst in (("wk", kt_sb), ("wq", qt_sb)):
                for m in range(2):
                    fs.append(
                        lambda wname=wname, dst=dst, m=m: qk_group(wname, dst, m)
                    )

            def v_group(j):
                st = 4 * sc + j
                psvt = ps_proj.tile([128, 512], f32, tag="pst", name="pv")
                psv = psvt[:, 0:DO]
                for k in range(KT_N):
                    nc.tensor.matmul(
                        psv,
                        xs[k][:, j * 128 : (j + 1) * 128],
                        w_sb["wv"][:, k, :],
                        start=(k == 0),
                        stop=(k == KT_N - 1),
                    )
                nc.scalar.copy(
                    v_sb[st][:, :, 0:DH],
                    psv.rearrange("p (h d) -> p h d", h=HPC),
                )

            for j in range(4):
                fs.append(lambda j=j: v_group(j))
            return fs

        def emit_attention_S(qc, fillers=()):
            """S = QK^T, causal mask, softmax numerator P (fp16) + its
            transposes for q-tiles qc*4..qc*4+3, all heads. `fillers` are
            independent emission closures spread between the 16 S units to
            keep the in-order PE queue fed while the DVE/ACT softmax
            pipeline drains."""
            fillers = list(fillers)
            emitted = 0
            unit = 0
            pt_tiles = {}
            for h in range(HPC):
                pt_tiles[h] = ptrow_pool.tile(
                    [128, ST_N, 512], f16, tag=f"pt{h % 2}", name=f"pt{h % 2}"
                )
            for qt in range(qc * 4, qc * 4 + 4):
                L = (qt + 1) * 128
                for h in range(HPC):
                    m2, poff = h // 2, (h % 2) * 64
                    lhsT_q = qt_sb[m2][poff : poff + 64, qt * 128 : (qt + 1) * 128]
                    subs = [(0, min(L, SUB))]
                    if L > SUB:
                        subs.append((SUB, L - SUB))
                    mneg_parts = stats.tile([128, 2], f32, tag="mneg_p", name="mneg_p")
                    ps_tiles = []
                    for si, (off, ls) in enumerate(subs):
                        ps = ps_main.tile([128, SUB], f32, tag="srow", name="srow")
                        ps_tiles.append((ps, off, ls))
                        has_diag = off + ls == L
                        for c0 in range(0, ls, 512):
                            c1 = min(ls, c0 + 512)
                            if has_diag and c1 == ls:
                                # causal mask for the final chunk: seed the
                                # PSUM region with eye.T @ maskneg (triangle
                                # in the last 128 cols of the slice), then
                                # the S matmul accumulates onto it.
                                w = c1 - c0
                                nc.tensor.matmul(
                                    ps[:, c0:c1],
                                    eye_sb[:],
                                    mask_sb[:, 512 - w : 512],
                                    start=True,
                                    stop=False,
                                    skip_group_check=True,
                                )
                            nc.tensor.matmul(
                                ps[:, c0:c1],
                                lhsT_q,
                                kt_sb[m2][poff : poff + 64, off + c0 : off + c1],
                                start=not (has_diag and c1 == ls),
                                stop=True,
                                skip_group_check=True,
                            )
                        nc.vector.reduce_max(
                            mneg_parts[:, si : si + 1],
                            ps[:, :ls],
                            axis=mybir.AxisListType.X,
                            negate=True,
                        )
                    if len(subs) == 2:
                        mneg = stats.tile([128, 1], f32, tag="mneg", name="mneg")
                        nc.vector.tensor_reduce(
                            mneg[:, 0:1],
                            mneg_parts[:, 0:2],
                            axis=mybir.AxisListType.X,
                            op=mybir.AluOpType.min,
                        )
                        mneg_ap = mneg[:, 0:1]
                    else:
                        mneg_ap = mneg_parts[:, 0:1]

                    p_row = prow_pool.tile([128, SEQ], f16, tag="prow", name="prow")
                    for ps, off, ls in ps_tiles:
                        nc.scalar.activation(
                            p_row[:, off : off + ls],
                            ps[:, :ls],
                            mybir.ActivationFunctionType.Exp,
                            bias=mneg_ap,
                            scale=1.0,
                        )
                    # all transposes stay on ONE HWDGE queue: concurrent
                    # dma_start_transpose on two queues corrupts transfers
                    nc.sync.dma_start_transpose(
                        pt_tiles[h][
                            :, : qt + 1, (qt % 4) * 128 : (qt % 4) * 128 + 128
                        ],
                        p_row[:, :L],
                    )
                    unit += 1
                    want = (unit * len(fillers)) // 16
                    while emitted < want:
                        fillers[emitted]()
                        emitted += 1
            while emitted < len(fillers):
                fillers[emitted]()
                emitted += 1
            return pt_tiles

        def pv_head(qc, pt_tiles, h):
            po = ps_o.tile([65, 512], f32, tag="po", name="po")
            kt_hi = qc * 4 + 3
            for kt in range(kt_hi + 1):
                off = max(0, (kt - qc * 4)) * 128
                nc.tensor.matmul(
                    po[:, off:512],
                    v_sb[kt][:, h, :],
                    pt_tiles[h][:, kt, off:512],
                    start=(kt == 0),
                    stop=(kt == kt_hi),
                )
            ot = outp.tile([65, 512], f16, tag="ot", name="ot")
            nc.vector.tensor_copy(ot[:], po[:])
            nc.gpsimd.dma_start(outT[h, :, qc * 512 : (qc + 1) * 512], ot[:])

        def pv_fillers(qc, pt_tiles):
            return [
                lambda h=h: pv_head(qc, pt_tiles, h) for h in range(HPC)
            ]

        # software-pipelined emission: projections run two chunks ahead and
        # PV(i) lands inside S(i+2), spread between S units so the in-order
        # PE queue always has dispatchable matmuls.
        for f in proj_fillers(0):
            f()
        for f in proj_fillers(1):
            f()
        pts0 = emit_attention_S(0, proj_fillers(2))
        pts1 = emit_attention_S(1, proj_fillers(3))
        pts2 = emit_attention_S(2, pv_fillers(0, pts0))
        pts3 = emit_attention_S(3, pv_fillers(1, pts1))
        for f in pv_fillers(2, pts2):
            f()
        for f in pv_fillers(3, pts3):
            f()


def _guard_ldweights(nc):
    """Tile hoists standalone LDWEIGHTS prefetches for 2-byte matmuls but
    leaves the weight-readiness waits on the MATMUL — the LDW would read
    stale weights. Bacc's move_matmul_waits_to_ldweights only moves waits
    beyond the first, so a single-wait matmul still leaves its LDW
    unguarded. Move ALL waits from a matmul onto an immediately-preceding
    ldweights (they are emitted adjacent), which is strictly safe."""
    from concourse import mybir

    n = 0
    for f in nc.m.functions:
        for bb in f.blocks:
            prev = None
            for inst in bb.instructions:
                if (
                    type(inst).__name__ == "InstMatmult"
                    and prev is not None
                    and type(prev).__name__ == "InstLdweights"
                    and prev.engine == inst.engine
                ):
                    si = inst.sync_info
                    waits = list(si.on_wait) if si is not None else []
                    if waits:
                        psi = prev.sync_info
                        pw = list(psi.on_wait) if psi is not None else []
                        pu = list(psi.on_update) if psi is not None else []
                        prev.sync_info = mybir.SyncInfo(
                            on_wait=pw + waits, on_update=pu
                        )
                        inst.sync_info = mybir.SyncInfo(
                            on_wait=[], on_update=list(si.on_update)
                        )
                        n += 1
                prev = inst
    return n


def _split_waits(nc):
    """This container's walrus accepts at most ONE sync-wait per instruction
    on several opcodes ("Too many sync wait commands"). Hoist excess waits
    into standalone InstEventSemaphore instructions on the same engine."""
    from concourse import mybir

    n = 0
    for f in nc.m.functions:
        for bb in f.blocks:
            new = []
            for inst in list(bb.instructions):
                # Matmuls get ALL waits hoisted: walrus emits the fused
                # LDWEIGHTS before the matmul's own waits would fire, so a
                # wait left on the matmul does not guard the stationary
                # operand read. A standalone EventSemaphore before the
                # instruction always does.
                cap = 0 if type(inst).__name__ == "InstMatmult" else 1
                si = inst.sync_info
                waits = list(si.on_wait) if si is not None else []
                if len(waits) > cap:
                    for j, w in enumerate(waits[cap:]):
                        new.append(
                            mybir.InstEventSemaphore(
                                name=f"{inst.name}-w{j}",
                                engine=inst.engine,
                                ins=[],
                                outs=[],
                                sync_info=mybir.SyncInfo(on_wait=[w], on_update=[]),
                            )
                        )
                        n += 1
                    inst.sync_info = mybir.SyncInfo(
                        on_wait=waits[:cap], on_update=list(si.on_update)
                    )
                new.append(inst)
            bb.instructions = new
    return n


def _build_nc():
    import concourse.bass as bass
    import concourse.tile as tile
    from concourse import mybir

    f32r = mybir.dt.float32r
    f16 = mybir.dt.float16
    nc = bass.Bass(
        "TRN2",
        target_bir_lowering=False,
        debug=False,
        num_devices=NCORES,
    )
    # weights arrive host-transposed to [partition, k-tile, n] so the SBUF
    # load is one flat contiguous DMA
    wshape = [128, KT_N, DO]
    xT = nc.dram_tensor("xT", [DIN, SEQ], f32r, kind="ExternalInput").ap()
    wq_hi = nc.dram_tensor("wq_hi", wshape, f32r, kind="ExternalInput").ap()
    wq_lo = nc.dram_tensor("wq_lo", wshape, f32r, kind="ExternalInput").ap()
    wk_hi = nc.dram_tensor("wk_hi", wshape, f32r, kind="ExternalInput").ap()
    wk_lo = nc.dram_tensor("wk_lo", wshape, f32r, kind="ExternalInput").ap()
    wv = nc.dram_tensor("wv", wshape, f32r, kind="ExternalInput").ap()
    mask = nc.dram_tensor("mask", [128, 512], f16, kind="ExternalInput").ap()
    eye = nc.dram_tensor("eye", [128, 128], f16, kind="ExternalInput").ap()
    outT = nc.dram_tensor("outT", [HPC, DH + 1, SEQ], f16, kind="ExternalOutput").ap()

    with tile.TileContext(nc) as tc:
        _emit_core_kernel(
            tc, (outT,), (xT, wq_hi, wq_lo, wk_hi, wk_lo, wv, mask, eye)
        )
    _guard_ldweights(nc)
    _split_waits(nc)
    return nc


def make_mask():
    """Additive causal mask, fp16, [128, 512] with the 128x128 triangle in
    the last 128 columns — slicing mask[:, 512-w:512] puts the triangle at
    the end of any w-wide final chunk. -57344 is exactly representable and
    large enough that exp(s - 57344 - max) underflows to 0 for any score
    magnitude in this problem."""
    m = np.zeros((128, 512), dtype=np.float16)
    q = np.arange(128)[:, None]
    k = np.arange(128)[None, :]
    m[:, 384:512] = np.where(k > q, np.float16(-57344.0), np.float16(0.0))
    return m


def round_fp32r(a):
    """Round fp32 to the fp32r (E8M11) grid: RNE at the 12 dropped mantissa
    bits, matching the hardware's fp32_to_fp32r downconversion."""
    u = np.ascontiguousarray(a, dtype=np.float32).view(np.uint32)
    lsb = (u >> 12) & 1
    r = (u + 0x7FF + lsb) & 0xFFFFF000
    return r.view(np.float32)


def shard_inputs(x, W_q, W_k, W_v):
    x = np.asarray(x, dtype=np.float32)
    W_q = np.asarray(W_q, dtype=np.float32)
    W_k = np.asarray(W_k, dtype=np.float32)
    W_v = np.asarray(W_v, dtype=np.float32)
    mask = make_mask()
    eye = np.eye(128, dtype=np.float16)
    scale = 1.0 / math.sqrt(DH)
    wq_s = W_q * scale
    wq_hi = round_fp32r(wq_s)
    wq_lo = round_fp32r(wq_s - wq_hi)
    wk_hi = round_fp32r(W_k)
    wk_lo = round_fp32r(W_k - wk_hi)

    def pkn(w):
        """[DIN, DO_slice] -> [128, KT_N, DO_slice] (partition-major)."""
        return np.ascontiguousarray(
            w.reshape(KT_N, 128, w.shape[1]).transpose(1, 0, 2)
        )

    in_maps = []
    for c in range(NCORES):
        b, g = divmod(c, NCORES // B)
        sl = slice(g * DO, (g + 1) * DO)
        in_maps.append(
            {
                "xT": round_fp32r(x[b].T),
                "wq_hi": pkn(wq_hi[:, sl]),
                "wq_lo": pkn(wq_lo[:, sl]),
                "wk_hi": pkn(wk_hi[:, sl]),
                "wk_lo": pkn(wk_lo[:, sl]),
                "wv": pkn(round_fp32r(W_v[:, sl])),
                "mask": mask,
                "eye": eye,
            }
        )
    return in_maps


def assemble_output(results):
    out = np.zeros((B, SEQ, DIN), dtype=np.float32)
    for c in range(NCORES):
        b, g = divmod(c, NCORES // B)
        oT = np.asarray(results[c]["outT"], dtype=np.float32)  # [HPC, 65, SEQ]
        for h in range(HPC):
            col = g * DO + h * DH
            out[b, :, col : col + DH] = (oT[h, :DH, :] / oT[h, DH : DH + 1, :]).T
    return out


def _install_axon_ntff_hook():
    """Provide antenv.axon_hooks (missing in this image) so trace=True works
    under axon. Mirrors trn_agent_boot.trn_boot._ntff_profile_via_ctypes."""
    import contextlib
    import ctypes
    import sys
    import types

    if "antenv.axon_hooks" in sys.modules:
        return True
    try:
        lib = ctypes.CDLL("/opt/axon/libaxon_pjrt.so")
    except OSError:
        return False
    if not hasattr(lib, "axon_start_nrt_profile"):
        return False
    lib.axon_start_nrt_profile.argtypes = [
        ctypes.POINTER(ctypes.c_int64),
        ctypes.c_size_t,
    ]
    lib.axon_start_nrt_profile.restype = ctypes.c_int64
    lib.axon_stop_nrt_profile.argtypes = [ctypes.c_char_p]
    lib.axon_stop_nrt_profile.restype = ctypes.c_int64

    @contextlib.contextmanager
    def _hook(output_dir, device_ids):
        import jax

        jax.devices()
        if device_ids:
            ids = (ctypes.c_int64 * len(device_ids))(*device_ids)
            rc = lib.axon_start_nrt_profile(ids, len(device_ids))
        else:
            rc = lib.axon_start_nrt_profile(None, 0)
        if rc != 0:
            raise RuntimeError(f"axon_start_nrt_profile rc={rc}")
        try:
            yield
        finally:
            n = lib.axon_stop_nrt_profile(str(output_dir).encode())
            print(f"ntff profile: {n} file(s) written to {output_dir}")

    mod = types.ModuleType("antenv.axon_hooks")
    holder = [_hook]
    mod.get_axon_ntff_profile_hook = lambda: holder[0]
    mod.set_axon_ntff_profile_hook = lambda h: holder.__setitem__(0, h)
    sys.modules["antenv.axon_hooks"] = mod
    import antenv

    antenv.axon_hooks = mod
    return True


def kernel(x, W_q, W_k, W_v):
    global LAST_RESULTS
    import os

    import concourse.bass_utils as bass_utils
    from concourse.bass_utils import run_bass_kernel_spmd

    if "nc" not in _CACHE:
        _CACHE["nc"] = _build_nc()
    nc = _CACHE["nc"]

    in_maps = shard_inputs(x, W_q, W_k, W_v)

    trace = bool(int(os.environ.get("MHA_TRACE", "0")))
    if trace:
        trace = _install_axon_ntff_hook()
        # avoid the fish-bucket artifact upload in this container
        bass_utils.upload_artifacts = lambda d: str(d)
    res = run_bass_kernel_spmd(
        nc, in_maps, core_ids=list(range(NCORES)), trace=trace
    )
    LAST_RESULTS = res
    return assemble_output(res.results)

